# revision 1
# baseline (speedup 1.0000x reference)
"""DIVeR volume-rendering kernel for Trainium2 (Bass/Tile), 8-core SPMD.

Strategy: shard the 800x800 rays across 8 NeuronCores (100 image rows each),
replicate the voxel grid + MLP weights. Each core:
  - computes ray/AABB intersection + 9 sample points per ray (DVE, bit-exact
    index math vs the jax reference),
  - gathers 2x2x2 vertex features per sample point with indirect DMA
    (4 gathers of 32 contiguous f32 = the two z-neighbors, per point),
  - trilinear-blends on DVE, runs the tiny MLP on the tensor engine via
    block-diagonalized weights, composites on DVE/ACT,
  - writes its [80000, 3] slice back.
"""

import numpy as np

# problem constants (hardcoded per harness contract)
V = 128
D = 16
S = 8
HID = 32
H = W = 800
GRID = 2.0
VS = GRID / V          # 2^-6, exact
XMIN = -GRID / 2       # -1.0
NCORES = 8
P = 128                # partitions

NRAYS_CORE = H * W // NCORES   # 80000
RPP_FULL = NRAYS_CORE // P     # 625 rays per partition

# corner offsets in vertex-id space for (dx, dy) in {0,1}^2 (z pairs are
# contiguous and fetched inside the 32-float gather element)
C_OFF = [0, 129, 129 * 129, 129 * 129 + 129]


def _f32(x):
    return float(np.float32(x))


def build_program(ray_o_np, RPP=RPP_FULL, KB=25, SUB=5, dbg=False):
    """Build the Bass/Tile program for one core processing P*RPP rays.

    RPP: rays per partition (total);  KB: rays per partition per big-round;
    SUB: sub-rounds per big-round (gather granularity).
    Ray mapping: ray(p, b, k) = p*RPP + b*KB + k.
    """
    import contextlib

    import concourse.bass as bass
    import concourse.tile as tile
    from concourse import bacc, mybir
    from concourse.masks import make_identity

    AP = bass.AP
    dt = mybir.dt
    Alu = mybir.AluOpType
    Act = mybir.ActivationFunctionType
    Axis = mybir.AxisListType

    assert RPP % KB == 0 and KB % SUB == 0
    NB = RPP // KB          # big-rounds
    SUBK = KB // SUB        # rays/partition per sub-round
    NPTS = KB * (S + 1)     # sample points per partition per big-round
    NSEG = KB * S
    SPTS = SUBK * (S + 1)   # points per partition per sub-round
    nrays = P * RPP

    o_imm = [_f32(ray_o_np[i]) for i in range(3)]
    # fl(XMIN - o), fl(XMIN + GRID - o) in fp32, matching the reference
    tA = [float(np.float32(XMIN) - np.float32(o_imm[a])) for a in range(3)]
    tB = [float(np.float32(XMIN + GRID) - np.float32(o_imm[a]))
          for a in range(3)]
    fracs = [float(np.float32(s / S)) for s in range(S + 1)]

    nc = bacc.Bacc(
        "TRN2", target_bir_lowering=False, debug=False, enable_asserts=False
    )

    # ---- DRAM tensors ----
    featsD = nc.dram_tensor("feats", [129 * 129 * 129, 16], dt.float32,
                            kind="ExternalInput")
    maskD = nc.dram_tensor("mask", [V * V * V, 1], dt.uint8,
                           kind="ExternalInput")
    raydD = nc.dram_tensor("rayd", [nrays, 3], dt.float32,
                           kind="ExternalInput")
    # host-packed transposed dirs: [NB*3, KB*128]; row b*3 + a, col k*128 + p
    raydTD = nc.dram_tensor("raydT", [NB * 3, KB * 128], dt.float32,
                            kind="ExternalInput")
    w1bD = nc.dram_tensor("w1blk", [64, 128], dt.float32, kind="ExternalInput")
    w1dD = nc.dram_tensor("w1dir", [3, 128], dt.float32, kind="ExternalInput")
    w2bD = nc.dram_tensor("w2blk", [128, 16], dt.float32, kind="ExternalInput")
    b1rD = nc.dram_tensor("b1rep", [128, 1], dt.float32, kind="ExternalInput")
    b2mD = nc.dram_tensor("b2m", [128, 4], dt.float32, kind="ExternalInput")
    outD = nc.dram_tensor("out", [nrays, 3], dt.float32, kind="ExternalOutput")
    if dbg:
        NPTS0 = KB * (S + 1)
        dbgD = {
            "didx": nc.dram_tensor("didx", [128, NPTS0 * 4], dt.int32,
                                   kind="ExternalOutput"),
            "dFb": nc.dram_tensor("dFb", [128, NPTS0 * 16], dt.float32,
                                  kind="ExternalOutput"),
            "docc": nc.dram_tensor("docc", [128, KB * S], dt.uint8,
                                   kind="ExternalOutput"),
            "do_t": nc.dram_tensor("do_t", [128, KB * 32], dt.float32,
                                   kind="ExternalOutput"),
            "dC": nc.dram_tensor("dC", [128, (KB // SUB) * (S + 1) * 128],
                                 dt.float32, kind="ExternalOutput"),
            "dw8": nc.dram_tensor("dw8", [128, NPTS0 * 8], dt.float32,
                                  kind="ExternalOutput"),
            "dfeat": nc.dram_tensor("dfeat", [128, KB * S * 16], dt.float32,
                                    kind="ExternalOutput"),
            "dhts": nc.dram_tensor("dhts", [128, 256], dt.float32,
                                   kind="ExternalOutput"),
            "dwgt": nc.dram_tensor("dwgt", [128, KB * S], dt.float32,
                                   kind="ExternalOutput"),
        }

    def v(base, off, dims):
        """View into a tile AP: keep partition dim, override free dims.
        dims: list of [step, count] in elements. off in elements."""
        return AP(base.tensor, base.offset + off,
                  [list(base.ap[0])] + [list(d) for d in dims])

    with tile.TileContext(nc) as tc:
        ctx = contextlib.ExitStack()
        with ctx:
            cpool = ctx.enter_context(tc.tile_pool(name="consts", bufs=1))
            apool = ctx.enter_context(tc.tile_pool(name="stageA", bufs=1))
            gpool = ctx.enter_context(tc.tile_pool(name="gather", bufs=1))
            mpool = ctx.enter_context(tc.tile_pool(name="mlp", bufs=1))
            ppool = ctx.enter_context(
                tc.tile_pool(name="psum", bufs=2, space="PSUM"))
            opool = ctx.enter_context(tc.tile_pool(name="outp", bufs=1))

            # ---- constants ----
            ident = cpool.tile([128, 128], dt.float32, name="ident")
            make_identity(nc, ident[:])
            w1b = cpool.tile([64, 128], dt.float32, name="w1b")
            nc.sync.dma_start(w1b[:], w1bD.ap())
            w1d = cpool.tile([3, 128], dt.float32, name="w1d")
            nc.sync.dma_start(w1d[:], w1dD.ap())
            w2b = cpool.tile([128, 16], dt.float32, name="w2b")
            nc.sync.dma_start(w2b[:], w2bD.ap())
            b1r = cpool.tile([128, 1], dt.float32, name="b1r")
            nc.sync.dma_start(b1r[:], b1rD.ap())
            b2m = cpool.tile([128, 4], dt.float32, name="b2m")
            nc.sync.dma_start(b2m[:], b2mD.ap())
            epsT = cpool.tile([128, KB * 3], dt.float32, name="epsT")
            nc.vector.memset(epsT[:], 1e-9)

            out_sb = opool.tile([128, RPP * 3], dt.float32, name="out_sb")

            def A(name, free, dtype=dt.float32, tag=None, bufs=1, pool=None):
                return (pool or apool).tile(
                    [128, free], dtype, name=name, tag=tag or name.split("_")[0],
                    bufs=bufs)

            for b in range(NB):
                # ---- per-big-round input loads ----
                rd = A(f"rd_{b}", KB * 3, bufs=2)
                nc.sync.dma_start(
                    rd[:], AP(raydD, b * KB * 3,
                              [[RPP * 3, 128], [3, KB], [1, 3]]))
                rdT = apool.tile([3, KB * 128], dt.float32, name=f"rdT_{b}",
                                 tag="rdT", bufs=1)
                nc.sync.dma_start(
                    rdT[:], AP(raydTD, b * 3 * KB * 128,
                               [[KB * 128, 3], [1, KB * 128]]))

                # ---- stage A: ray setup ----
                absd = A(f"absd_{b}", KB * 3, tag="scr3a")
                nc.scalar.activation(absd[:], rd[:], Act.Abs)
                cgtf = A(f"cgtf_{b}", KB * 3, tag="scr3b")
                nc.vector.tensor_scalar(cgtf[:], absd[:], 1e-9, None,
                                        Alu.is_gt)
                cgt = A(f"cgt_{b}", KB * 3, dtype=dt.uint8, tag="scr3e")
                nc.vector.tensor_copy(cgt[:], cgtf[:])
                sd = A(f"sd_{b}", KB * 3)
                nc.vector.select(sd[:], cgt[:], rd[:], epsT[:])
                inv = A(f"inv_{b}", KB * 3)
                nc.vector.reciprocal(inv[:], sd[:])

                # t0/t1 per axis into axis-major blocks [3, KB]
                t0 = A(f"t0_{b}", 3 * KB, tag="scr3a")
                t1 = A(f"t1_{b}", 3 * KB, tag="scr3b")
                for a in range(3):
                    inva = v(inv[:], a, [[3, KB]])
                    nc.vector.tensor_scalar(
                        v(t0[:], a * KB, [[1, KB]]), inva, tA[a], None,
                        Alu.mult)
                    nc.vector.tensor_scalar(
                        v(t1[:], a * KB, [[1, KB]]), inva, tB[a], None,
                        Alu.mult)
                lo = A(f"lo_{b}", 3 * KB, tag="scr3c")
                nc.vector.tensor_tensor(lo[:], t0[:], t1[:], op=Alu.min)
                hi = A(f"hi_{b}", 3 * KB, tag="scr3d")
                nc.vector.tensor_tensor(hi[:], t0[:], t1[:], op=Alu.max)

                tn = A(f"tn_{b}", KB)
                nc.vector.tensor_tensor(
                    tn[:], lo[:, 0:KB], lo[:, KB:2 * KB], op=Alu.max)
                nc.vector.tensor_tensor(
                    tn[:], tn[:], lo[:, 2 * KB:3 * KB], op=Alu.max)
                nc.vector.tensor_scalar(tn[:], tn[:], 0.0, None, Alu.max)
                tf = A(f"tf_{b}", KB)
                nc.vector.tensor_tensor(
                    tf[:], hi[:, 0:KB], hi[:, KB:2 * KB], op=Alu.min)
                nc.vector.tensor_tensor(
                    tf[:], tf[:], hi[:, 2 * KB:3 * KB], op=Alu.min)

                delta = A(f"delta_{b}", KB)
                nc.vector.tensor_tensor(delta[:], tf[:], tn[:],
                                        op=Alu.subtract)
                valid = A(f"valid_{b}", KB, bufs=2)
                nc.vector.tensor_tensor(valid[:], tf[:], tn[:], op=Alu.is_gt)
                seg = A(f"seg_{b}", KB, bufs=2)
                nc.vector.tensor_scalar(seg[:], delta[:], _f32(1.0 / S), None,
                                        Alu.mult)
                hseg = A(f"hseg_{b}", KB)
                nc.vector.tensor_scalar(hseg[:], delta[:], _f32(0.5 / S),
                                        None, Alu.mult)

                # ts [128, KB*(S+1)] layout (k, s)
                ts = A(f"ts_{b}", NPTS)
                for s in range(S + 1):
                    nc.vector.scalar_tensor_tensor(
                        v(ts[:], s, [[S + 1, KB]]), delta[:], fracs[s], tn[:],
                        op0=Alu.mult, op1=Alu.add)

                # g per axis: g = clip((pt+1)*64, 0, 128); floor; frac
                i0 = []
                fr = []
                for a in range(3):
                    m = A(f"m{a}_{b}", NPTS, tag="scrP0")
                    dbc = v(rd[:], a, [[3, KB], [0, S + 1]])
                    tsv = v(ts[:], 0, [[S + 1, KB], [1, S + 1]])
                    nc.vector.tensor_tensor(m[:], tsv, dbc, op=Alu.mult)
                    g = A(f"g{a}_{b}", NPTS, tag=f"gP{a}")
                    nc.vector.tensor_scalar(g[:], m[:], o_imm[a], 1.0,
                                            Alu.add, Alu.add)
                    nc.vector.tensor_scalar(g[:], g[:], 64.0, 0.0,
                                            Alu.mult, Alu.max)
                    nc.vector.tensor_scalar(g[:], g[:], 128.0, None, Alu.min)
                    ri = A(f"ri{a}_{b}", NPTS, dtype=dt.int32, tag="scrPi")
                    nc.vector.tensor_copy(ri[:], g[:])
                    rf = A(f"rf{a}_{b}", NPTS, tag="scrP1")
                    nc.vector.tensor_copy(rf[:], ri[:])
                    cg = A(f"cg{a}_{b}", NPTS, tag="scrP2")
                    nc.vector.tensor_tensor(cg[:], rf[:], g[:], op=Alu.is_gt)
                    i0a = A(f"i0{a}_{b}", NPTS, tag=f"i0P{a}")
                    nc.vector.scalar_tensor_tensor(
                        i0a[:], cg[:], -1.0, rf[:], op0=Alu.mult, op1=Alu.add)
                    nc.vector.tensor_scalar(i0a[:], i0a[:], 127.0, None,
                                            Alu.min)
                    fa = A(f"f{a}_{b}", NPTS, tag=f"fP{a}")
                    nc.vector.tensor_tensor(fa[:], g[:], i0a[:],
                                            op=Alu.subtract)
                    i0.append(i0a)
                    fr.append(fa)

                vid = A(f"vid_{b}", NPTS, tag="scrP0")
                nc.vector.scalar_tensor_tensor(
                    vid[:], i0[0][:], 129.0, i0[1][:], op0=Alu.mult,
                    op1=Alu.add)
                nc.vector.scalar_tensor_tensor(
                    vid[:], vid[:], 129.0, i0[2][:], op0=Alu.mult, op1=Alu.add)
                idx4f = A(f"idx4f_{b}", NPTS * 4, tag="scrP6")
                for c in range(4):
                    nc.vector.tensor_scalar(
                        v(idx4f[:], c, [[4, NPTS]]), vid[:], float(C_OFF[c]),
                        None, Alu.add)
                idx4 = A(f"idx4_{b}", NPTS * 4, dtype=dt.int32, bufs=2)
                nc.vector.tensor_copy(idx4[:], idx4f[:])
                if dbg and b == 0:
                    nc.sync.dma_start(dbgD['didx'].ap(), idx4[:])

                # ---- blend weights w8 [128, NPTS*8], (c*2+z) minor ----
                wx0 = A(f"wx0_{b}", NPTS, tag="scrP1")
                nc.vector.tensor_scalar(wx0[:], fr[0][:], -1.0, 1.0,
                                        Alu.mult, Alu.add)
                wy0 = A(f"wy0_{b}", NPTS, tag="scrP2")
                nc.vector.tensor_scalar(wy0[:], fr[1][:], -1.0, 1.0,
                                        Alu.mult, Alu.add)
                hsb = v(hseg[:], 0, [[1, KB], [0, S + 1]])
                wz1 = A(f"wz1_{b}", NPTS, tag="scrP3")
                nc.vector.tensor_tensor(wz1[:], fr[2][:], hsb, op=Alu.mult)
                wz0 = A(f"wz0_{b}", NPTS, tag="scrP4")
                nc.vector.tensor_tensor(wz0[:], hsb, wz1[:], op=Alu.subtract)
                w8 = A(f"w8_{b}", NPTS * 8)
                axyt = A(f"axy_{b}", NPTS * 4, tag="scrP5")
                pairs = [(0, wx0, 0, wy0), (0, wx0, 1, fr[1]),
                         (1, fr[0], 0, wy0), (1, fr[0], 1, fr[1])]
                for c, (dx, wxa, dy, wya) in enumerate(pairs):
                    av = v(axyt[:], c, [[4, NPTS]])
                    nc.vector.tensor_tensor(av, wxa[:], wya[:], op=Alu.mult)
                for c in range(4):
                    av = v(axyt[:], c, [[4, NPTS]])
                    for z, wza in ((0, wz0), (1, wz1)):
                        nc.vector.tensor_tensor(
                            v(w8[:], c * 2 + z, [[8, NPTS]]),
                            av, wza[:], op=Alu.mult)

                if dbg and b == 0:
                    nc.sync.dma_start(dbgD['dw8'].ap(), w8[:])
                # ---- midpoints / occupancy indices ----
                tm = A(f"tm_{b}", NSEG, tag="scrS0")
                nc.vector.scalar_tensor_tensor(
                    v(tm[:], 0, [[S, KB], [1, S]]),
                    v(ts[:], 0, [[S + 1, KB], [1, S]]), 1.0,
                    v(ts[:], 1, [[S + 1, KB], [1, S]]),
                    op0=Alu.bypass, op1=Alu.add)
                nc.vector.tensor_scalar(tm[:], tm[:], 0.5, None, Alu.mult)
                vim = []
                for a in range(3):
                    mm = A(f"mm{a}_{b}", NSEG, tag="scrS1")
                    dbc = v(rd[:], a, [[3, KB], [0, S]])
                    tmv = v(tm[:], 0, [[S, KB], [1, S]])
                    nc.vector.tensor_tensor(mm[:], tmv, dbc, op=Alu.mult)
                    nc.vector.tensor_scalar(mm[:], mm[:], o_imm[a], 1.0,
                                            Alu.add, Alu.add)
                    nc.vector.tensor_scalar(mm[:], mm[:], 64.0, 0.0,
                                            Alu.mult, Alu.max)
                    nc.vector.tensor_scalar(mm[:], mm[:], 128.0, None,
                                            Alu.min)
                    ri = A(f"mri{a}_{b}", NSEG, dtype=dt.int32, tag="scrSi")
                    nc.vector.tensor_copy(ri[:], mm[:])
                    rf = A(f"mrf{a}_{b}", NSEG, tag="scrS2")
                    nc.vector.tensor_copy(rf[:], ri[:])
                    cg = A(f"mcg{a}_{b}", NSEG, tag="scrS3")
                    nc.vector.tensor_tensor(cg[:], rf[:], mm[:], op=Alu.is_gt)
                    flo = A(f"mflo{a}_{b}", NSEG, tag=f"viP{a}")
                    nc.vector.scalar_tensor_tensor(
                        flo[:], cg[:], -1.0, rf[:], op0=Alu.mult, op1=Alu.add)
                    nc.vector.tensor_scalar(flo[:], flo[:], 127.0, None,
                                            Alu.min)
                    vim.append(flo)
                vidm = A(f"vidm_{b}", NSEG, tag="scrS1")
                nc.vector.scalar_tensor_tensor(
                    vidm[:], vim[0][:], 128.0, vim[1][:], op0=Alu.mult,
                    op1=Alu.add)
                nc.vector.scalar_tensor_tensor(
                    vidm[:], vidm[:], 128.0, vim[2][:], op0=Alu.mult,
                    op1=Alu.add)
                vidmi = A(f"vidmi_{b}", NSEG, dtype=dt.int32, bufs=2)
                nc.vector.tensor_copy(vidmi[:], vidm[:])
                occ8 = A(f"occ8_{b}", NSEG, dtype=dt.uint8, bufs=2)
                for t in range(NSEG):
                    nc.gpsimd.indirect_dma_start(
                        out=v(occ8[:], t, [[1, 1]]), out_offset=None,
                        in_=maskD.ap(),
                        in_offset=bass.IndirectOffsetOnAxis(
                            ap=vidmi[:, t:t + 1], axis=0))
                if dbg and b == 0:
                    nc.sync.dma_start(dbgD['docc'].ap(), occ8[:])

                # ---- gather + blend per sub-round ----
                Fb = A(f"Fb_{b}", NPTS * 16)
                for sr in range(SUB):
                    C = gpool.tile([128, SPTS * 128], dt.float32,
                                   name=f"C_{b}_{sr}", tag="C", bufs=2)
                    # HW indirect DMA consumes ONE offset per partition
                    # (verified on-device); one instruction per 32-elem run.
                    for t in range(SPTS * 4):
                        nc.gpsimd.indirect_dma_start(
                            out=v(C[:], t * 32, [[1, 32]]), out_offset=None,
                            in_=featsD.ap(),
                            in_offset=bass.IndirectOffsetOnAxis(
                                ap=idx4[:, sr * SPTS * 4 + t:
                                        sr * SPTS * 4 + t + 1],
                                axis=0))
                    if dbg and b == 0 and sr == 0:
                        nc.sync.dma_start(dbgD['dC'].ap(), C[:])
                    wc = gpool.tile([128, SPTS * 128], dt.float32,
                                    name=f"wc_{b}_{sr}", tag="wc")
                    for cz in range(8):
                        nc.vector.scalar_tensor_tensor(
                            v(wc[:], cz * 16, [[128, SPTS], [1, 16]]),
                            v(C[:], cz * 16, [[128, SPTS], [1, 16]]), 1.0,
                            v(w8[:], sr * SPTS * 8 + cz, [[8, SPTS], [0, 16]]),
                            op0=Alu.bypass, op1=Alu.mult)
                    r1 = gpool.tile([128, SPTS * 64], dt.float32,
                                    name=f"r1_{b}_{sr}", tag="r1")
                    for cp in range(2):
                        nc.vector.scalar_tensor_tensor(
                            v(r1[:], cp * 32, [[64, SPTS], [1, 32]]),
                            v(wc[:], cp * 64, [[128, SPTS], [1, 32]]), 1.0,
                            v(wc[:], cp * 64 + 32, [[128, SPTS], [1, 32]]),
                            op0=Alu.bypass, op1=Alu.add)
                    r2 = gpool.tile([128, SPTS * 32], dt.float32,
                                    name=f"r2_{b}_{sr}", tag="r2")
                    nc.vector.scalar_tensor_tensor(
                        r2[:], v(r1[:], 0, [[64, SPTS], [1, 32]]), 1.0,
                        v(r1[:], 32, [[64, SPTS], [1, 32]]),
                        op0=Alu.bypass, op1=Alu.add)
                    nc.vector.scalar_tensor_tensor(
                        Fb[:, sr * SPTS * 16:(sr + 1) * SPTS * 16],
                        v(r2[:], 0, [[32, SPTS], [1, 16]]), 1.0,
                        v(r2[:], 16, [[32, SPTS], [1, 16]]),
                        op0=Alu.bypass, op1=Alu.add)

                # trapezoid (0.5*seg already folded into w8)
                feat = mpool.tile([128, NSEG * 16], dt.float32,
                                  name=f"feat_{b}", tag="feat")
                nc.vector.scalar_tensor_tensor(
                    feat[:],
                    v(Fb[:], 0, [[(S + 1) * 16, KB], [1, S * 16]]), 1.0,
                    v(Fb[:], 16, [[(S + 1) * 16, KB], [1, S * 16]]),
                    op0=Alu.bypass, op1=Alu.add)

                if dbg and b == 0:
                    nc.sync.dma_start(dbgD['dFb'].ap(), Fb[:])
                    nc.sync.dma_start(dbgD['dfeat'].ap(), feat[:])
                # ---- MLP per chunk k ----
                o_t = mpool.tile([128, KB * 32], dt.float32, name=f"o_{b}",
                                 tag="o", bufs=2)
                for k in range(KB):
                    psXT = ppool.tile([64, 256], dt.float32,
                                      name=f"psXT_{b}_{k}", tag="psXT")
                    for hh in range(2):
                        nc.tensor.transpose(
                            psXT[:, hh * 128:(hh + 1) * 128],
                            feat[:, k * 128 + hh * 64:k * 128 + (hh + 1) * 64],
                            ident[:])
                    xts = mpool.tile([64, 256], dt.float32,
                                     name=f"xts_{b}_{k}", tag="xts", bufs=2)
                    nc.scalar.copy(xts[:], psXT[:])
                    psH = ppool.tile([128, 256], dt.float32,
                                     name=f"psH_{b}_{k}", tag="psH")
                    for hh in range(2):
                        nc.tensor.matmul(
                            psH[:, hh * 128:(hh + 1) * 128], lhsT=w1b[:],
                            rhs=xts[:, hh * 128:(hh + 1) * 128],
                            start=True, stop=False)
                        nc.tensor.matmul(
                            psH[:, hh * 128:(hh + 1) * 128], lhsT=w1d[:],
                            rhs=rdT[:, k * 128:(k + 1) * 128],
                            start=False, stop=True)
                    hts = mpool.tile([128, 256], dt.float32,
                                     name=f"hts_{b}_{k}", tag="hts", bufs=2)
                    if dbg and b == 0 and k == 0:
                        pass
                    nc.scalar.activation(hts[:], psH[:], Act.Relu,
                                         bias=b1r[:], scale=1.0)
                    if dbg and b == 0 and k == 0:
                        nc.sync.dma_start(dbgD['dhts'].ap(), hts[:])
                    psO = ppool.tile([128, 32], dt.float32,
                                     name=f"psO_{b}_{k}", tag="psO")
                    for hh in range(2):
                        nc.tensor.matmul(
                            psO[:, hh * 16:(hh + 1) * 16],
                            lhsT=hts[:, hh * 128:(hh + 1) * 128], rhs=w2b[:],
                            start=True, stop=True)
                    b2v = v(b2m[:], 0, [[0, 8], [1, 4]])
                    nc.vector.scalar_tensor_tensor(
                        v(o_t[:], k * 32, [[4, 8], [1, 4]]),
                        v(psO[:], 0, [[4, 8], [1, 4]]), 1.0, b2v,
                        op0=Alu.bypass, op1=Alu.add)

                # ---- compositing ----
                if dbg and b == 0:
                    nc.sync.dma_start(dbgD['do_t'].ap(), o_t[:])
                ov_sig = v(o_t[:], 0, [[32, KB], [4, S]])
                # softplus(x) = ln(1 + exp(x)) (no Softplus table on this arch)
                spe = A(f"spe_{b}", NSEG, tag="scrS7", pool=mpool)
                nc.scalar.activation(spe[:], ov_sig, Act.Exp)
                sp = A(f"sp_{b}", NSEG, tag="scrS2", pool=mpool)
                nc.scalar.activation(sp[:], spe[:], Act.Ln, bias=1.0)
                occf = A(f"occf_{b}", NSEG, tag="scrS3", pool=mpool)
                nc.vector.tensor_copy(occf[:], occ8[:])
                vbc = v(valid[:], 0, [[1, KB], [0, S]])
                nc.vector.scalar_tensor_tensor(
                    occf[:], occf[:], 1.0, vbc, op0=Alu.bypass, op1=Alu.mult)
                sig = A(f"sig_{b}", NSEG, tag="scrS4", pool=mpool)
                nc.vector.tensor_tensor(sig[:], sp[:], occf[:], op=Alu.mult)
                segb = v(seg[:], 0, [[1, KB], [0, S]])
                nc.vector.tensor_tensor(sig[:], sig[:], segb, op=Alu.mult)
                ee = A(f"ee_{b}", NSEG, tag="scrS5", pool=mpool)
                nc.scalar.activation(ee[:], sig[:], Act.Exp, scale=-1.0)
                alpha = A(f"alpha_{b}", NSEG, tag="alphaP", pool=mpool)
                nc.vector.tensor_scalar(alpha[:], ee[:], -1.0, 1.0,
                                        Alu.mult, Alu.add)
                am1 = A(f"am1_{b}", NSEG, tag="scrS2", pool=mpool)
                nc.vector.tensor_scalar(am1[:], alpha[:], -1.0, 1.0,
                                        Alu.mult, Alu.add)
                nc.vector.tensor_scalar(am1[:], am1[:], 1e-10, None, Alu.add)
                # inclusive cumprod along s (log-steps)
                T1 = A(f"T1_{b}", NSEG, tag="scrS3", pool=mpool)
                nc.vector.tensor_copy(v(T1[:], 0, [[S, KB]]),
                                      v(am1[:], 0, [[S, KB]]))
                nc.vector.tensor_tensor(
                    v(T1[:], 1, [[S, KB], [1, S - 1]]),
                    v(am1[:], 1, [[S, KB], [1, S - 1]]),
                    v(am1[:], 0, [[S, KB], [1, S - 1]]), op=Alu.mult)
                T2 = A(f"T2_{b}", NSEG, tag="scrS6", pool=mpool)
                nc.vector.tensor_copy(v(T2[:], 0, [[S, KB], [1, 2]]),
                                      v(T1[:], 0, [[S, KB], [1, 2]]))
                nc.vector.tensor_tensor(
                    v(T2[:], 2, [[S, KB], [1, S - 2]]),
                    v(T1[:], 2, [[S, KB], [1, S - 2]]),
                    v(T1[:], 0, [[S, KB], [1, S - 2]]), op=Alu.mult)
                T3 = A(f"T3_{b}", NSEG, tag="scrS2", pool=mpool)
                nc.vector.tensor_copy(v(T3[:], 0, [[S, KB], [1, 4]]),
                                      v(T2[:], 0, [[S, KB], [1, 4]]))
                nc.vector.tensor_tensor(
                    v(T3[:], 4, [[S, KB], [1, S - 4]]),
                    v(T2[:], 4, [[S, KB], [1, S - 4]]),
                    v(T2[:], 0, [[S, KB], [1, S - 4]]), op=Alu.mult)
                # wgt = Te * alpha (Te = shifted inclusive cumprod)
                wgt = A(f"wgt_{b}", NSEG, tag="scrS3", pool=mpool)
                nc.vector.tensor_copy(v(wgt[:], 0, [[S, KB]]),
                                      v(alpha[:], 0, [[S, KB]]))
                nc.vector.tensor_tensor(
                    v(wgt[:], 1, [[S, KB], [1, S - 1]]),
                    v(T3[:], 0, [[S, KB], [1, S - 1]]),
                    v(alpha[:], 1, [[S, KB], [1, S - 1]]), op=Alu.mult)
                if dbg and b == 0:
                    nc.sync.dma_start(dbgD['dwgt'].ap(), wgt[:])
                # rgb & accumulation (channel-blocked layout: (ch, k, s))
                rgb = A(f"rgb_{b}", NSEG * 3, tag="rgbP", pool=mpool)
                for ch in range(3):
                    nc.scalar.activation(
                        rgb[:, ch * NSEG:(ch + 1) * NSEG],
                        v(o_t[:], 1 + ch, [[32, KB], [4, S]]), Act.Sigmoid)
                wrgb = A(f"wrgb_{b}", NSEG * 3, tag="wrgbP", pool=mpool)
                for ch in range(3):
                    nc.vector.scalar_tensor_tensor(
                        wrgb[:, ch * NSEG:(ch + 1) * NSEG],
                        rgb[:, ch * NSEG:(ch + 1) * NSEG], 1.0,
                        v(wgt[:], 0, [[S, KB], [1, S]]),
                        op0=Alu.bypass, op1=Alu.mult)
                col = A(f"col_{b}", KB * 3, tag="colP", pool=mpool)
                nc.vector.tensor_reduce(
                    col[:], v(wrgb[:], 0, [[NSEG, 3], [S, KB], [1, S]]),
                    axis=Axis.X, op=Alu.add)
                acc = A(f"acc_{b}", KB, tag="accP", pool=mpool)
                nc.vector.tensor_reduce(
                    acc[:], v(wgt[:], 0, [[S, KB], [1, S]]),
                    axis=Axis.X, op=Alu.add)
                u = A(f"u_{b}", KB, tag="uP", pool=mpool)
                nc.vector.tensor_scalar(u[:], acc[:], -1.0, 1.0,
                                        Alu.mult, Alu.add)
                # col is (ch, k); out_sb wants (k, ch)
                nc.vector.scalar_tensor_tensor(
                    v(out_sb[:], b * KB * 3, [[3, KB], [1, 3]]),
                    v(col[:], 0, [[1, KB], [KB, 3]]), 1.0,
                    v(u[:], 0, [[1, KB], [0, 3]]),
                    op0=Alu.bypass, op1=Alu.add)

            # final store
            nc.sync.dma_start(
                AP(outD, 0, [[RPP * 3, 128], [1, RPP * 3]]), out_sb[:])

    nc.compile()
    return nc


def make_host_inputs(inputs, core, RPP=RPP_FULL, KB=25):
    """Build the per-core in_map (host-side data layout prep only)."""
    nrays = P * RPP
    NB = RPP // KB
    vf = np.ascontiguousarray(
        np.asarray(inputs["voxel_feats"], dtype=np.float32).reshape(-1, 16))
    vm = np.ascontiguousarray(
        np.asarray(inputs["voxel_mask"]).astype(np.uint8).reshape(-1, 1))
    rd_full = np.asarray(inputs["ray_d"], dtype=np.float32).reshape(-1, 3)
    rd_c = np.ascontiguousarray(rd_full[core * nrays:(core + 1) * nrays])
    # raydT[b*3 + a, k*128 + p] = rd_c[p*RPP + b*KB + k, a]
    rdT = rd_c.reshape(P, NB, KB, 3).transpose(1, 3, 2, 0)
    rdT = np.ascontiguousarray(rdT.reshape(NB * 3, KB * P))
    w1 = np.asarray(inputs["w1"], dtype=np.float32)
    w2 = np.asarray(inputs["w2"], dtype=np.float32)
    b1 = np.asarray(inputs["b1"], dtype=np.float32)
    b2 = np.asarray(inputs["b2"], dtype=np.float32)
    w1blk = np.zeros((64, 128), np.float32)
    for s in range(4):
        w1blk[s * 16:(s + 1) * 16, s * 32:(s + 1) * 32] = w1[:16]
    w1dir = np.zeros((3, 128), np.float32)
    for s in range(4):
        w1dir[:, s * 32:(s + 1) * 32] = w1[16:19]
    w2blk = np.zeros((128, 16), np.float32)
    for s in range(4):
        w2blk[s * 32:(s + 1) * 32, s * 4:(s + 1) * 4] = w2
    b1rep = np.tile(b1, 4).reshape(128, 1).astype(np.float32)
    b2m = np.tile(b2.reshape(1, 4), (128, 1)).astype(np.float32)
    return {
        "feats": vf, "mask": vm, "rayd": rd_c, "raydT": rdT,
        "w1blk": w1blk, "w1dir": w1dir, "w2blk": np.ascontiguousarray(w2blk),
        "b1rep": b1rep, "b2m": b2m,
    }


_nc_cache = {}


def kernel(trace=False, **inputs):
    """Full-input, full-output entry point. Shards across 8 NeuronCores."""
    from concourse.bass_utils import run_bass_kernel_spmd

    ray_o = np.asarray(inputs["ray_o"], dtype=np.float32)
    key = tuple(ray_o.tolist())
    if key not in _nc_cache:
        _nc_cache[key] = build_program(ray_o)
    nc = _nc_cache[key]

    in_maps = [make_host_inputs(inputs, c) for c in range(NCORES)]
    res = run_bass_kernel_spmd(nc, in_maps, core_ids=list(range(NCORES)),
                               trace=trace)
    out = np.concatenate([r["out"] for r in res.results], axis=0)
    out = out.reshape(H, W, 3).astype(np.float32)
    kernel._last_results = res
    return out



# revision 2
# speedup vs baseline: 2.5412x; 2.5412x over previous
"""DIVeR volume-rendering kernel for Trainium2 (Bass/Tile), 8-core SPMD.

Strategy: shard the 800x800 rays across 8 NeuronCores (100 image rows each),
replicate the voxel grid + MLP weights. Each core:
  - computes ray/AABB intersection + 9 sample points per ray (DVE, bit-exact
    index math vs the jax reference),
  - gathers the full 2x2x2 vertex-feature record per sample point with ONE
    indirect DMA of 128 contiguous f32 (512B) from a host-relayouted
    8-corner record table [128^3, 128] (SWDGE dispatch overhead is ~1us
    per indirect instruction regardless of payload, so fewer/fatter
    gathers are the main lever: 4x fewer than per-corner-pair gathers),
  - trilinear-blends on DVE, runs the tiny MLP on the tensor engine via
    block-diagonalized weights, composites on DVE/ACT,
  - writes its [80000, 3] slice back.
"""

import numpy as np

# problem constants (hardcoded per harness contract)
V = 128
D = 16
S = 8
HID = 32
H = W = 800
GRID = 2.0
VS = GRID / V          # 2^-6, exact
XMIN = -GRID / 2       # -1.0
NCORES = 8
P = 128                # partitions

NRAYS_CORE = H * W // NCORES   # 80000
RPP_FULL = NRAYS_CORE // P     # 625 rays per partition


def _f32(x):
    return float(np.float32(x))


def build_program(ray_o_np, RPP=RPP_FULL, KB=25, SUB=5, dbg=False):
    """Build the Bass/Tile program for one core processing P*RPP rays.

    RPP: rays per partition (total);  KB: rays per partition per big-round;
    SUB: sub-rounds per big-round (gather granularity).
    Ray mapping: ray(p, b, k) = p*RPP + b*KB + k.
    """
    import contextlib

    import concourse.bass as bass
    import concourse.tile as tile
    from concourse import bacc, mybir
    from concourse.masks import make_identity

    AP = bass.AP
    dt = mybir.dt
    Alu = mybir.AluOpType
    Act = mybir.ActivationFunctionType
    Axis = mybir.AxisListType

    assert RPP % KB == 0 and KB % SUB == 0
    NB = RPP // KB          # big-rounds
    SUBK = KB // SUB        # rays/partition per sub-round
    NPTS = KB * (S + 1)     # sample points per partition per big-round
    NSEG = KB * S
    SPTS = SUBK * (S + 1)   # points per partition per sub-round
    nrays = P * RPP

    o_imm = [_f32(ray_o_np[i]) for i in range(3)]
    # fl(XMIN - o), fl(XMIN + GRID - o) in fp32, matching the reference
    tA = [float(np.float32(XMIN) - np.float32(o_imm[a])) for a in range(3)]
    tB = [float(np.float32(XMIN + GRID) - np.float32(o_imm[a]))
          for a in range(3)]
    fracs = [float(np.float32(s / S)) for s in range(S + 1)]

    nc = bacc.Bacc(
        "TRN2", target_bir_lowering=False, debug=False, enable_asserts=False
    )

    # ---- DRAM tensors ----
    # 8-corner record table: rec[(x*128+y)*128+z] = the 4 z-pair runs
    # (dx,dy) in [(0,0),(0,1),(1,0),(1,1)], each 32 f32.
    recD = nc.dram_tensor("rec", [V * V * V, 128], dt.float32,
                          kind="ExternalInput")
    maskD = nc.dram_tensor("mask", [V * V * V, 1], dt.uint8,
                           kind="ExternalInput")
    raydD = nc.dram_tensor("rayd", [nrays, 3], dt.float32,
                           kind="ExternalInput")
    # host-packed transposed dirs: [NB*3, KB*128]; row b*3 + a, col k*128 + p
    raydTD = nc.dram_tensor("raydT", [NB * 3, KB * 128], dt.float32,
                            kind="ExternalInput")
    w1bD = nc.dram_tensor("w1blk", [64, 128], dt.float32, kind="ExternalInput")
    w1dD = nc.dram_tensor("w1dir", [3, 128], dt.float32, kind="ExternalInput")
    w2bD = nc.dram_tensor("w2blk", [128, 16], dt.float32, kind="ExternalInput")
    b1rD = nc.dram_tensor("b1rep", [128, 1], dt.float32, kind="ExternalInput")
    b2mD = nc.dram_tensor("b2m", [128, 4], dt.float32, kind="ExternalInput")
    outD = nc.dram_tensor("out", [nrays, 3], dt.float32, kind="ExternalOutput")
    if dbg:
        NPTS0 = KB * (S + 1)
        dbgD = {
            "didx": nc.dram_tensor("didx", [128, NPTS0], dt.int32,
                                   kind="ExternalOutput"),
            "dFb": nc.dram_tensor("dFb", [128, NPTS0 * 16], dt.float32,
                                  kind="ExternalOutput"),
            "docc": nc.dram_tensor("docc", [128, KB * S], dt.uint8,
                                   kind="ExternalOutput"),
            "do_t": nc.dram_tensor("do_t", [128, KB * 32], dt.float32,
                                   kind="ExternalOutput"),
            "dC": nc.dram_tensor("dC", [128, (KB // SUB) * (S + 1) * 128],
                                 dt.float32, kind="ExternalOutput"),
            "dw8": nc.dram_tensor("dw8", [128, NPTS0 * 8], dt.float32,
                                  kind="ExternalOutput"),
            "dfeat": nc.dram_tensor("dfeat", [128, KB * S * 16], dt.float32,
                                    kind="ExternalOutput"),
            "dhts": nc.dram_tensor("dhts", [128, 256], dt.float32,
                                   kind="ExternalOutput"),
            "dwgt": nc.dram_tensor("dwgt", [128, KB * S], dt.float32,
                                   kind="ExternalOutput"),
        }

    def v(base, off, dims):
        """View into a tile AP: keep partition dim, override free dims.
        dims: list of [step, count] in elements. off in elements."""
        return AP(base.tensor, base.offset + off,
                  [list(base.ap[0])] + [list(d) for d in dims])

    with tile.TileContext(nc) as tc:
        ctx = contextlib.ExitStack()
        with ctx:
            cpool = ctx.enter_context(tc.tile_pool(name="consts", bufs=1))
            apool = ctx.enter_context(tc.tile_pool(name="stageA", bufs=1))
            gpool = ctx.enter_context(tc.tile_pool(name="gather", bufs=1))
            mpool = ctx.enter_context(tc.tile_pool(name="mlp", bufs=1))
            ppool = ctx.enter_context(
                tc.tile_pool(name="psum", bufs=2, space="PSUM"))
            opool = ctx.enter_context(tc.tile_pool(name="outp", bufs=1))

            # ---- constants ----
            ident = cpool.tile([128, 128], dt.float32, name="ident")
            make_identity(nc, ident[:])
            w1b = cpool.tile([64, 128], dt.float32, name="w1b")
            nc.sync.dma_start(w1b[:], w1bD.ap())
            w1d = cpool.tile([3, 128], dt.float32, name="w1d")
            nc.sync.dma_start(w1d[:], w1dD.ap())
            w2b = cpool.tile([128, 16], dt.float32, name="w2b")
            nc.sync.dma_start(w2b[:], w2bD.ap())
            b1r = cpool.tile([128, 1], dt.float32, name="b1r")
            nc.sync.dma_start(b1r[:], b1rD.ap())
            b2m = cpool.tile([128, 4], dt.float32, name="b2m")
            nc.sync.dma_start(b2m[:], b2mD.ap())
            epsT = cpool.tile([128, KB * 3], dt.float32, name="epsT")
            nc.vector.memset(epsT[:], 1e-9)

            out_sb = opool.tile([128, RPP * 3], dt.float32, name="out_sb")

            def A(name, free, dtype=dt.float32, tag=None, bufs=1, pool=None):
                return (pool or apool).tile(
                    [128, free], dtype, name=name, tag=tag or name.split("_")[0],
                    bufs=bufs)

            for b in range(NB):
                # ---- per-big-round input loads ----
                rd = A(f"rd_{b}", KB * 3, bufs=2)
                nc.sync.dma_start(
                    rd[:], AP(raydD, b * KB * 3,
                              [[RPP * 3, 128], [3, KB], [1, 3]]))
                rdT = apool.tile([3, KB * 128], dt.float32, name=f"rdT_{b}",
                                 tag="rdT", bufs=1)
                nc.sync.dma_start(
                    rdT[:], AP(raydTD, b * 3 * KB * 128,
                               [[KB * 128, 3], [1, KB * 128]]))

                # ---- stage A: ray setup ----
                absd = A(f"absd_{b}", KB * 3, tag="scr3a")
                nc.scalar.activation(absd[:], rd[:], Act.Abs)
                cgtf = A(f"cgtf_{b}", KB * 3, tag="scr3b")
                nc.vector.tensor_scalar(cgtf[:], absd[:], 1e-9, None,
                                        Alu.is_gt)
                cgt = A(f"cgt_{b}", KB * 3, dtype=dt.uint8, tag="scr3e")
                nc.vector.tensor_copy(cgt[:], cgtf[:])
                sd = A(f"sd_{b}", KB * 3)
                nc.vector.select(sd[:], cgt[:], rd[:], epsT[:])
                inv = A(f"inv_{b}", KB * 3)
                nc.vector.reciprocal(inv[:], sd[:])

                # t0/t1 per axis into axis-major blocks [3, KB]
                t0 = A(f"t0_{b}", 3 * KB, tag="scr3a")
                t1 = A(f"t1_{b}", 3 * KB, tag="scr3b")
                for a in range(3):
                    inva = v(inv[:], a, [[3, KB]])
                    nc.vector.tensor_scalar(
                        v(t0[:], a * KB, [[1, KB]]), inva, tA[a], None,
                        Alu.mult)
                    nc.vector.tensor_scalar(
                        v(t1[:], a * KB, [[1, KB]]), inva, tB[a], None,
                        Alu.mult)
                lo = A(f"lo_{b}", 3 * KB, tag="scr3c")
                nc.vector.tensor_tensor(lo[:], t0[:], t1[:], op=Alu.min)
                hi = A(f"hi_{b}", 3 * KB, tag="scr3d")
                nc.vector.tensor_tensor(hi[:], t0[:], t1[:], op=Alu.max)

                tn = A(f"tn_{b}", KB)
                nc.vector.tensor_tensor(
                    tn[:], lo[:, 0:KB], lo[:, KB:2 * KB], op=Alu.max)
                nc.vector.tensor_tensor(
                    tn[:], tn[:], lo[:, 2 * KB:3 * KB], op=Alu.max)
                nc.vector.tensor_scalar(tn[:], tn[:], 0.0, None, Alu.max)
                tf = A(f"tf_{b}", KB)
                nc.vector.tensor_tensor(
                    tf[:], hi[:, 0:KB], hi[:, KB:2 * KB], op=Alu.min)
                nc.vector.tensor_tensor(
                    tf[:], tf[:], hi[:, 2 * KB:3 * KB], op=Alu.min)

                delta = A(f"delta_{b}", KB)
                nc.vector.tensor_tensor(delta[:], tf[:], tn[:],
                                        op=Alu.subtract)
                valid = A(f"valid_{b}", KB, bufs=2)
                nc.vector.tensor_tensor(valid[:], tf[:], tn[:], op=Alu.is_gt)
                seg = A(f"seg_{b}", KB, bufs=2)
                nc.vector.tensor_scalar(seg[:], delta[:], _f32(1.0 / S), None,
                                        Alu.mult)
                hseg = A(f"hseg_{b}", KB)
                nc.vector.tensor_scalar(hseg[:], delta[:], _f32(0.5 / S),
                                        None, Alu.mult)

                # ts [128, KB*(S+1)] layout (k, s)
                ts = A(f"ts_{b}", NPTS)
                for s in range(S + 1):
                    nc.vector.scalar_tensor_tensor(
                        v(ts[:], s, [[S + 1, KB]]), delta[:], fracs[s], tn[:],
                        op0=Alu.mult, op1=Alu.add)

                # g per axis: g = clip((pt+1)*64, 0, 128); floor; frac
                i0 = []
                fr = []
                for a in range(3):
                    m = A(f"m{a}_{b}", NPTS, tag="scrP0")
                    dbc = v(rd[:], a, [[3, KB], [0, S + 1]])
                    tsv = v(ts[:], 0, [[S + 1, KB], [1, S + 1]])
                    nc.vector.tensor_tensor(m[:], tsv, dbc, op=Alu.mult)
                    g = A(f"g{a}_{b}", NPTS, tag=f"gP{a}")
                    nc.vector.tensor_scalar(g[:], m[:], o_imm[a], 1.0,
                                            Alu.add, Alu.add)
                    nc.vector.tensor_scalar(g[:], g[:], 64.0, 0.0,
                                            Alu.mult, Alu.max)
                    nc.vector.tensor_scalar(g[:], g[:], 128.0, None, Alu.min)
                    ri = A(f"ri{a}_{b}", NPTS, dtype=dt.int32, tag="scrPi")
                    nc.vector.tensor_copy(ri[:], g[:])
                    rf = A(f"rf{a}_{b}", NPTS, tag="scrP1")
                    nc.vector.tensor_copy(rf[:], ri[:])
                    cg = A(f"cg{a}_{b}", NPTS, tag="scrP2")
                    nc.vector.tensor_tensor(cg[:], rf[:], g[:], op=Alu.is_gt)
                    i0a = A(f"i0{a}_{b}", NPTS, tag=f"i0P{a}")
                    nc.vector.scalar_tensor_tensor(
                        i0a[:], cg[:], -1.0, rf[:], op0=Alu.mult, op1=Alu.add)
                    nc.vector.tensor_scalar(i0a[:], i0a[:], 127.0, None,
                                            Alu.min)
                    fa = A(f"f{a}_{b}", NPTS, tag=f"fP{a}")
                    nc.vector.tensor_tensor(fa[:], g[:], i0a[:],
                                            op=Alu.subtract)
                    i0.append(i0a)
                    fr.append(fa)

                # record index: ((x*128)+y)*128+z  (fits fp32 exactly)
                vid = A(f"vid_{b}", NPTS, tag="scrP0")
                nc.vector.scalar_tensor_tensor(
                    vid[:], i0[0][:], 128.0, i0[1][:], op0=Alu.mult,
                    op1=Alu.add)
                nc.vector.scalar_tensor_tensor(
                    vid[:], vid[:], 128.0, i0[2][:], op0=Alu.mult, op1=Alu.add)
                idx1 = A(f"idx1_{b}", NPTS, dtype=dt.int32, bufs=2)
                nc.vector.tensor_copy(idx1[:], vid[:])
                if dbg and b == 0:
                    nc.sync.dma_start(dbgD['didx'].ap(), idx1[:])

                # ---- blend weights w8 [128, NPTS*8], (c*2+z) minor ----
                wx0 = A(f"wx0_{b}", NPTS, tag="scrP1")
                nc.vector.tensor_scalar(wx0[:], fr[0][:], -1.0, 1.0,
                                        Alu.mult, Alu.add)
                wy0 = A(f"wy0_{b}", NPTS, tag="scrP2")
                nc.vector.tensor_scalar(wy0[:], fr[1][:], -1.0, 1.0,
                                        Alu.mult, Alu.add)
                hsb = v(hseg[:], 0, [[1, KB], [0, S + 1]])
                wz1 = A(f"wz1_{b}", NPTS, tag="scrP3")
                nc.vector.tensor_tensor(wz1[:], fr[2][:], hsb, op=Alu.mult)
                wz0 = A(f"wz0_{b}", NPTS, tag="scrP4")
                nc.vector.tensor_tensor(wz0[:], hsb, wz1[:], op=Alu.subtract)
                w8 = A(f"w8_{b}", NPTS * 8)
                axyt = A(f"axy_{b}", NPTS * 4, tag="scrP5")
                pairs = [(0, wx0, 0, wy0), (0, wx0, 1, fr[1]),
                         (1, fr[0], 0, wy0), (1, fr[0], 1, fr[1])]
                for c, (dx, wxa, dy, wya) in enumerate(pairs):
                    av = v(axyt[:], c, [[4, NPTS]])
                    nc.vector.tensor_tensor(av, wxa[:], wya[:], op=Alu.mult)
                for c in range(4):
                    av = v(axyt[:], c, [[4, NPTS]])
                    for z, wza in ((0, wz0), (1, wz1)):
                        nc.vector.tensor_tensor(
                            v(w8[:], c * 2 + z, [[8, NPTS]]),
                            av, wza[:], op=Alu.mult)

                if dbg and b == 0:
                    nc.sync.dma_start(dbgD['dw8'].ap(), w8[:])
                # ---- midpoints / occupancy indices ----
                tm = A(f"tm_{b}", NSEG, tag="scrS0")
                nc.vector.scalar_tensor_tensor(
                    v(tm[:], 0, [[S, KB], [1, S]]),
                    v(ts[:], 0, [[S + 1, KB], [1, S]]), 1.0,
                    v(ts[:], 1, [[S + 1, KB], [1, S]]),
                    op0=Alu.bypass, op1=Alu.add)
                nc.vector.tensor_scalar(tm[:], tm[:], 0.5, None, Alu.mult)
                vim = []
                for a in range(3):
                    mm = A(f"mm{a}_{b}", NSEG, tag="scrS1")
                    dbc = v(rd[:], a, [[3, KB], [0, S]])
                    tmv = v(tm[:], 0, [[S, KB], [1, S]])
                    nc.vector.tensor_tensor(mm[:], tmv, dbc, op=Alu.mult)
                    nc.vector.tensor_scalar(mm[:], mm[:], o_imm[a], 1.0,
                                            Alu.add, Alu.add)
                    nc.vector.tensor_scalar(mm[:], mm[:], 64.0, 0.0,
                                            Alu.mult, Alu.max)
                    nc.vector.tensor_scalar(mm[:], mm[:], 128.0, None,
                                            Alu.min)
                    ri = A(f"mri{a}_{b}", NSEG, dtype=dt.int32, tag="scrSi")
                    nc.vector.tensor_copy(ri[:], mm[:])
                    rf = A(f"mrf{a}_{b}", NSEG, tag="scrS2")
                    nc.vector.tensor_copy(rf[:], ri[:])
                    cg = A(f"mcg{a}_{b}", NSEG, tag="scrS3")
                    nc.vector.tensor_tensor(cg[:], rf[:], mm[:], op=Alu.is_gt)
                    flo = A(f"mflo{a}_{b}", NSEG, tag=f"viP{a}")
                    nc.vector.scalar_tensor_tensor(
                        flo[:], cg[:], -1.0, rf[:], op0=Alu.mult, op1=Alu.add)
                    nc.vector.tensor_scalar(flo[:], flo[:], 127.0, None,
                                            Alu.min)
                    vim.append(flo)
                vidm = A(f"vidm_{b}", NSEG, tag="scrS1")
                nc.vector.scalar_tensor_tensor(
                    vidm[:], vim[0][:], 128.0, vim[1][:], op0=Alu.mult,
                    op1=Alu.add)
                nc.vector.scalar_tensor_tensor(
                    vidm[:], vidm[:], 128.0, vim[2][:], op0=Alu.mult,
                    op1=Alu.add)
                vidmi = A(f"vidmi_{b}", NSEG, dtype=dt.int32, bufs=2)
                nc.vector.tensor_copy(vidmi[:], vidm[:])
                occ8 = A(f"occ8_{b}", NSEG, dtype=dt.uint8, bufs=2)
                for t in range(NSEG):
                    nc.gpsimd.indirect_dma_start(
                        out=v(occ8[:], t, [[1, 1]]), out_offset=None,
                        in_=maskD.ap(),
                        in_offset=bass.IndirectOffsetOnAxis(
                            ap=vidmi[:, t:t + 1], axis=0))
                if dbg and b == 0:
                    nc.sync.dma_start(dbgD['docc'].ap(), occ8[:])

                # ---- gather + blend per sub-round ----
                Fb = A(f"Fb_{b}", NPTS * 16)
                for sr in range(SUB):
                    C = gpool.tile([128, SPTS * 128], dt.float32,
                                   name=f"C_{b}_{sr}", tag="C", bufs=2)
                    # ONE indirect DMA per point: 128 contiguous f32 (the
                    # whole 8-corner record). HW consumes one offset per
                    # partition and streams the out-AP length from there.
                    for t in range(SPTS):
                        nc.gpsimd.indirect_dma_start(
                            out=v(C[:], t * 128, [[1, 128]]), out_offset=None,
                            in_=recD.ap(),
                            in_offset=bass.IndirectOffsetOnAxis(
                                ap=idx1[:, sr * SPTS + t:
                                        sr * SPTS + t + 1],
                                axis=0))
                    if dbg and b == 0 and sr == 0:
                        nc.sync.dma_start(dbgD['dC'].ap(), C[:])
                    wc = gpool.tile([128, SPTS * 128], dt.float32,
                                    name=f"wc_{b}_{sr}", tag="wc")
                    for cz in range(8):
                        nc.vector.scalar_tensor_tensor(
                            v(wc[:], cz * 16, [[128, SPTS], [1, 16]]),
                            v(C[:], cz * 16, [[128, SPTS], [1, 16]]), 1.0,
                            v(w8[:], sr * SPTS * 8 + cz, [[8, SPTS], [0, 16]]),
                            op0=Alu.bypass, op1=Alu.mult)
                    r1 = gpool.tile([128, SPTS * 64], dt.float32,
                                    name=f"r1_{b}_{sr}", tag="r1")
                    for cp in range(2):
                        nc.vector.scalar_tensor_tensor(
                            v(r1[:], cp * 32, [[64, SPTS], [1, 32]]),
                            v(wc[:], cp * 64, [[128, SPTS], [1, 32]]), 1.0,
                            v(wc[:], cp * 64 + 32, [[128, SPTS], [1, 32]]),
                            op0=Alu.bypass, op1=Alu.add)
                    r2 = gpool.tile([128, SPTS * 32], dt.float32,
                                    name=f"r2_{b}_{sr}", tag="r2")
                    nc.vector.scalar_tensor_tensor(
                        r2[:], v(r1[:], 0, [[64, SPTS], [1, 32]]), 1.0,
                        v(r1[:], 32, [[64, SPTS], [1, 32]]),
                        op0=Alu.bypass, op1=Alu.add)
                    nc.vector.scalar_tensor_tensor(
                        Fb[:, sr * SPTS * 16:(sr + 1) * SPTS * 16],
                        v(r2[:], 0, [[32, SPTS], [1, 16]]), 1.0,
                        v(r2[:], 16, [[32, SPTS], [1, 16]]),
                        op0=Alu.bypass, op1=Alu.add)

                # trapezoid (0.5*seg already folded into w8)
                feat = mpool.tile([128, NSEG * 16], dt.float32,
                                  name=f"feat_{b}", tag="feat")
                nc.vector.scalar_tensor_tensor(
                    feat[:],
                    v(Fb[:], 0, [[(S + 1) * 16, KB], [1, S * 16]]), 1.0,
                    v(Fb[:], 16, [[(S + 1) * 16, KB], [1, S * 16]]),
                    op0=Alu.bypass, op1=Alu.add)

                if dbg and b == 0:
                    nc.sync.dma_start(dbgD['dFb'].ap(), Fb[:])
                    nc.sync.dma_start(dbgD['dfeat'].ap(), feat[:])
                # ---- MLP per chunk k ----
                o_t = mpool.tile([128, KB * 32], dt.float32, name=f"o_{b}",
                                 tag="o", bufs=2)
                for k in range(KB):
                    psXT = ppool.tile([64, 256], dt.float32,
                                      name=f"psXT_{b}_{k}", tag="psXT")
                    for hh in range(2):
                        nc.tensor.transpose(
                            psXT[:, hh * 128:(hh + 1) * 128],
                            feat[:, k * 128 + hh * 64:k * 128 + (hh + 1) * 64],
                            ident[:])
                    xts = mpool.tile([64, 256], dt.float32,
                                     name=f"xts_{b}_{k}", tag="xts", bufs=2)
                    nc.scalar.copy(xts[:], psXT[:])
                    psH = ppool.tile([128, 256], dt.float32,
                                     name=f"psH_{b}_{k}", tag="psH")
                    for hh in range(2):
                        nc.tensor.matmul(
                            psH[:, hh * 128:(hh + 1) * 128], lhsT=w1b[:],
                            rhs=xts[:, hh * 128:(hh + 1) * 128],
                            start=True, stop=False)
                        nc.tensor.matmul(
                            psH[:, hh * 128:(hh + 1) * 128], lhsT=w1d[:],
                            rhs=rdT[:, k * 128:(k + 1) * 128],
                            start=False, stop=True)
                    hts = mpool.tile([128, 256], dt.float32,
                                     name=f"hts_{b}_{k}", tag="hts", bufs=2)
                    nc.scalar.activation(hts[:], psH[:], Act.Relu,
                                         bias=b1r[:], scale=1.0)
                    if dbg and b == 0 and k == 0:
                        nc.sync.dma_start(dbgD['dhts'].ap(), hts[:])
                    psO = ppool.tile([128, 32], dt.float32,
                                     name=f"psO_{b}_{k}", tag="psO")
                    for hh in range(2):
                        nc.tensor.matmul(
                            psO[:, hh * 16:(hh + 1) * 16],
                            lhsT=hts[:, hh * 128:(hh + 1) * 128], rhs=w2b[:],
                            start=True, stop=True)
                    b2v = v(b2m[:], 0, [[0, 8], [1, 4]])
                    nc.vector.scalar_tensor_tensor(
                        v(o_t[:], k * 32, [[4, 8], [1, 4]]),
                        v(psO[:], 0, [[4, 8], [1, 4]]), 1.0, b2v,
                        op0=Alu.bypass, op1=Alu.add)

                # ---- compositing ----
                if dbg and b == 0:
                    nc.sync.dma_start(dbgD['do_t'].ap(), o_t[:])
                ov_sig = v(o_t[:], 0, [[32, KB], [4, S]])
                # softplus(x) = ln(1 + exp(x)) (no Softplus table on this arch)
                spe = A(f"spe_{b}", NSEG, tag="scrS7", pool=mpool)
                nc.scalar.activation(spe[:], ov_sig, Act.Exp)
                sp = A(f"sp_{b}", NSEG, tag="scrS2", pool=mpool)
                nc.scalar.activation(sp[:], spe[:], Act.Ln, bias=1.0)
                occf = A(f"occf_{b}", NSEG, tag="scrS3", pool=mpool)
                nc.vector.tensor_copy(occf[:], occ8[:])
                vbc = v(valid[:], 0, [[1, KB], [0, S]])
                nc.vector.scalar_tensor_tensor(
                    occf[:], occf[:], 1.0, vbc, op0=Alu.bypass, op1=Alu.mult)
                sig = A(f"sig_{b}", NSEG, tag="scrS4", pool=mpool)
                nc.vector.tensor_tensor(sig[:], sp[:], occf[:], op=Alu.mult)
                segb = v(seg[:], 0, [[1, KB], [0, S]])
                nc.vector.tensor_tensor(sig[:], sig[:], segb, op=Alu.mult)
                ee = A(f"ee_{b}", NSEG, tag="scrS5", pool=mpool)
                nc.scalar.activation(ee[:], sig[:], Act.Exp, scale=-1.0)
                alpha = A(f"alpha_{b}", NSEG, tag="alphaP", pool=mpool)
                nc.vector.tensor_scalar(alpha[:], ee[:], -1.0, 1.0,
                                        Alu.mult, Alu.add)
                am1 = A(f"am1_{b}", NSEG, tag="scrS2", pool=mpool)
                nc.vector.tensor_scalar(am1[:], alpha[:], -1.0, 1.0,
                                        Alu.mult, Alu.add)
                nc.vector.tensor_scalar(am1[:], am1[:], 1e-10, None, Alu.add)
                # inclusive cumprod along s (log-steps)
                T1 = A(f"T1_{b}", NSEG, tag="scrS3", pool=mpool)
                nc.vector.tensor_copy(v(T1[:], 0, [[S, KB]]),
                                      v(am1[:], 0, [[S, KB]]))
                nc.vector.tensor_tensor(
                    v(T1[:], 1, [[S, KB], [1, S - 1]]),
                    v(am1[:], 1, [[S, KB], [1, S - 1]]),
                    v(am1[:], 0, [[S, KB], [1, S - 1]]), op=Alu.mult)
                T2 = A(f"T2_{b}", NSEG, tag="scrS6", pool=mpool)
                nc.vector.tensor_copy(v(T2[:], 0, [[S, KB], [1, 2]]),
                                      v(T1[:], 0, [[S, KB], [1, 2]]))
                nc.vector.tensor_tensor(
                    v(T2[:], 2, [[S, KB], [1, S - 2]]),
                    v(T1[:], 2, [[S, KB], [1, S - 2]]),
                    v(T1[:], 0, [[S, KB], [1, S - 2]]), op=Alu.mult)
                T3 = A(f"T3_{b}", NSEG, tag="scrS2", pool=mpool)
                nc.vector.tensor_copy(v(T3[:], 0, [[S, KB], [1, 4]]),
                                      v(T2[:], 0, [[S, KB], [1, 4]]))
                nc.vector.tensor_tensor(
                    v(T3[:], 4, [[S, KB], [1, S - 4]]),
                    v(T2[:], 4, [[S, KB], [1, S - 4]]),
                    v(T2[:], 0, [[S, KB], [1, S - 4]]), op=Alu.mult)
                # wgt = Te * alpha (Te = shifted inclusive cumprod)
                wgt = A(f"wgt_{b}", NSEG, tag="scrS3", pool=mpool)
                nc.vector.tensor_copy(v(wgt[:], 0, [[S, KB]]),
                                      v(alpha[:], 0, [[S, KB]]))
                nc.vector.tensor_tensor(
                    v(wgt[:], 1, [[S, KB], [1, S - 1]]),
                    v(T3[:], 0, [[S, KB], [1, S - 1]]),
                    v(alpha[:], 1, [[S, KB], [1, S - 1]]), op=Alu.mult)
                if dbg and b == 0:
                    nc.sync.dma_start(dbgD['dwgt'].ap(), wgt[:])
                # rgb & accumulation (channel-blocked layout: (ch, k, s))
                rgb = A(f"rgb_{b}", NSEG * 3, tag="rgbP", pool=mpool)
                for ch in range(3):
                    nc.scalar.activation(
                        rgb[:, ch * NSEG:(ch + 1) * NSEG],
                        v(o_t[:], 1 + ch, [[32, KB], [4, S]]), Act.Sigmoid)
                wrgb = A(f"wrgb_{b}", NSEG * 3, tag="wrgbP", pool=mpool)
                for ch in range(3):
                    nc.vector.scalar_tensor_tensor(
                        wrgb[:, ch * NSEG:(ch + 1) * NSEG],
                        rgb[:, ch * NSEG:(ch + 1) * NSEG], 1.0,
                        v(wgt[:], 0, [[S, KB], [1, S]]),
                        op0=Alu.bypass, op1=Alu.mult)
                col = A(f"col_{b}", KB * 3, tag="colP", pool=mpool)
                nc.vector.tensor_reduce(
                    col[:], v(wrgb[:], 0, [[NSEG, 3], [S, KB], [1, S]]),
                    axis=Axis.X, op=Alu.add)
                acc = A(f"acc_{b}", KB, tag="accP", pool=mpool)
                nc.vector.tensor_reduce(
                    acc[:], v(wgt[:], 0, [[S, KB], [1, S]]),
                    axis=Axis.X, op=Alu.add)
                u = A(f"u_{b}", KB, tag="uP", pool=mpool)
                nc.vector.tensor_scalar(u[:], acc[:], -1.0, 1.0,
                                        Alu.mult, Alu.add)
                # col is (ch, k); out_sb wants (k, ch)
                nc.vector.scalar_tensor_tensor(
                    v(out_sb[:], b * KB * 3, [[3, KB], [1, 3]]),
                    v(col[:], 0, [[1, KB], [KB, 3]]), 1.0,
                    v(u[:], 0, [[1, KB], [0, 3]]),
                    op0=Alu.bypass, op1=Alu.add)

            # final store
            nc.sync.dma_start(
                AP(outD, 0, [[RPP * 3, 128], [1, RPP * 3]]), out_sb[:])

    nc.compile()
    return nc


_rec_cache = {}


def _build_record_table(voxel_feats):
    """8-corner record table [V^3, 128] f32 (host layout prep only).
    rec[(x*128+y)*128+z] = concat over (dx,dy) in [(0,0),(0,1),(1,0),(1,1)]
    of [feats[x+dx,y+dy,z], feats[x+dx,y+dy,z+1]]."""
    key = id(voxel_feats)
    if key in _rec_cache:
        return _rec_cache[key]
    f = np.asarray(voxel_feats, dtype=np.float32)
    blocks = []
    for dx, dy in [(0, 0), (0, 1), (1, 0), (1, 1)]:
        blk = f[dx:dx + V, dy:dy + V, :, :]
        b2 = np.concatenate([blk[:, :, :V, :], blk[:, :, 1:V + 1, :]],
                            axis=-1)
        blocks.append(b2)
    rec = np.ascontiguousarray(
        np.concatenate(blocks, axis=-1).reshape(V * V * V, 128))
    _rec_cache.clear()
    _rec_cache[key] = rec
    return rec


def make_host_inputs(inputs, core, rec, RPP=RPP_FULL, KB=25):
    """Build the per-core in_map (host-side data layout prep only)."""
    nrays = P * RPP
    NB = RPP // KB
    vm = np.ascontiguousarray(
        np.asarray(inputs["voxel_mask"]).astype(np.uint8).reshape(-1, 1))
    rd_full = np.asarray(inputs["ray_d"], dtype=np.float32).reshape(-1, 3)
    rd_c = np.ascontiguousarray(rd_full[core * nrays:(core + 1) * nrays])
    # raydT[b*3 + a, k*128 + p] = rd_c[p*RPP + b*KB + k, a]
    rdT = rd_c.reshape(P, NB, KB, 3).transpose(1, 3, 2, 0)
    rdT = np.ascontiguousarray(rdT.reshape(NB * 3, KB * P))
    w1 = np.asarray(inputs["w1"], dtype=np.float32)
    w2 = np.asarray(inputs["w2"], dtype=np.float32)
    b1 = np.asarray(inputs["b1"], dtype=np.float32)
    b2 = np.asarray(inputs["b2"], dtype=np.float32)
    w1blk = np.zeros((64, 128), np.float32)
    for s in range(4):
        w1blk[s * 16:(s + 1) * 16, s * 32:(s + 1) * 32] = w1[:16]
    w1dir = np.zeros((3, 128), np.float32)
    for s in range(4):
        w1dir[:, s * 32:(s + 1) * 32] = w1[16:19]
    w2blk = np.zeros((128, 16), np.float32)
    for s in range(4):
        w2blk[s * 32:(s + 1) * 32, s * 4:(s + 1) * 4] = w2
    b1rep = np.tile(b1, 4).reshape(128, 1).astype(np.float32)
    b2m = np.tile(b2.reshape(1, 4), (128, 1)).astype(np.float32)
    return {
        "rec": rec, "mask": vm, "rayd": rd_c, "raydT": rdT,
        "w1blk": w1blk, "w1dir": w1dir, "w2blk": np.ascontiguousarray(w2blk),
        "b1rep": b1rep, "b2m": b2m,
    }


_nc_cache = {}


def kernel(trace=False, **inputs):
    """Full-input, full-output entry point. Shards across 8 NeuronCores."""
    from concourse.bass_utils import run_bass_kernel_spmd

    ray_o = np.asarray(inputs["ray_o"], dtype=np.float32)
    key = tuple(ray_o.tolist())
    if key not in _nc_cache:
        _nc_cache[key] = build_program(ray_o)
    nc = _nc_cache[key]

    rec = _build_record_table(inputs["voxel_feats"])
    in_maps = [make_host_inputs(inputs, c, rec) for c in range(NCORES)]
    res = run_bass_kernel_spmd(nc, in_maps, core_ids=list(range(NCORES)),
                               trace=trace)
    out = np.concatenate([r["out"] for r in res.results], axis=0)
    out = out.reshape(H, W, 3).astype(np.float32)
    kernel._last_results = res
    return out
